# revision 1
# baseline (speedup 1.0000x reference)
"""GCN (2-layer + MLP head) on 8 NeuronCores — v3: indirect-DMA gather +
matmul aggregation.

Per core (nodes dst-sharded, 12500 real / 12800 padded):
  GEMM: hw = (x @ W1) * dinv  (node-major, fp32, SBUF-resident self table)
  slice [12800, 64] bf16 -> AllGather -> table [102400, 64] bf16 in DRAM.
  Edge tokens dst-sorted, grouped by dst-block (128 dsts), each block's
  token count padded to a cross-core-common multiple of 128.
  gpsimd.indirect_dma_start gathers 8192 tokens/call from the full table
  (HW DGE, int32 row indices, token-major out [128, 64, 64] bf16).
  Aggregation: per token-tile, M[t,d] = (dstrel[t]==d) built on DVE
  (is_equal vs iota, bf16), PE matmul M^T @ g accumulates a dst-block's
  tiles in PSUM; one fold per block into SBUF acc (fp32).
  Pointwise: h = relu((acc + hw_self)*dinv + b); layer-2 GEMM via PE
  transpose of h; head MLP per tile; out [2, 12800] per core.
"""
import numpy as np

import concourse.bacc as bacc
import concourse.mybir as mybir
from concourse import bass
from concourse.tile import TileContext
from concourse.bass_utils import run_bass_kernel_spmd
from concourse.masks import make_identity

N = 100000
NS_RAW = 12500
NS = 12800
NTILE = NS // 128          # 100
NBLK = 98                  # blocks containing real dsts
IN_CH, HID, HID2, OUT = 256, 64, 32, 2
NT_CALL = 64               # gather-call size in token-tiles (8192 tokens)
PAD_SENT = -1000.0

_compiled = {}


def _build_schedule(src, dst):
    """Token schedule, shape-equalized across cores.

    Returns (per_core, tbs):
      per_core[c] = (idx32 [128, TOTC] int32 table-row indices,
                     dstrel [128, TOTC] f32 dst-local-in-block / PAD_SENT)
      tbs = [tiles per block] (common across cores), sum = TOTC
    """
    core = dst // NS_RAW
    dstl = (dst % NS_RAW).astype(np.int64)
    src_core = src // NS_RAW
    src_local = (src % NS_RAW).astype(np.int64)

    # Per-core node permutation: sort own dsts by in-degree desc, deal
    # round-robin into the 98 blocks so per-block token counts are nearly
    # equal (minimizes cross-core common padding). perm[old_local] = new.
    perms = []
    for c in range(8):
        degc = np.bincount(dstl[core == c], minlength=NS_RAW)
        order = np.argsort(-degc, kind="stable")
        ranks = np.arange(NS_RAW)
        newpos = (ranks % NBLK) * 128 + ranks // NBLK
        perm = np.empty(NS_RAW, np.int64)
        perm[order] = newpos
        perms.append(perm)
    permsA = np.stack(perms)                       # [8, NS_RAW]

    dstp = permsA[core, dstl]                      # permuted dst-local
    rows = src_core * NS + permsA[src_core, src_local]  # permuted table row
    blk = dstp // 128

    cells = {}
    ntile = np.zeros((8, NBLK), np.int64)
    for c in range(8):
        m = core == c
        o = np.argsort(dstp[m], kind="stable")
        rr, dd, bb = rows[m][o], dstp[m][o], blk[m][o]
        bounds = np.searchsorted(bb, np.arange(NBLK + 1))
        cells[c] = (bounds, rr, dd)
        ntile[c] = np.ceil((bounds[1:] - bounds[:-1]) / 128).astype(np.int64)

    tbs = ntile.max(axis=0)            # tiles per block, common
    assert tbs.min() >= 1
    TOTC = int(tbs.sum())

    per_core = []
    for c in range(8):
        bounds, rr, dd = cells[c]
        r_parts, d_parts = [], []
        for b in range(NBLK):
            lo, hi = bounds[b], bounds[b + 1]
            n = hi - lo
            cap = int(tbs[b]) * 128
            r_parts.append(rr[lo:hi])
            d_parts.append(dd[lo:hi] - 128 * b)
            if cap > n:
                r_parts.append(np.zeros(cap - n, np.int64))
                d_parts.append(np.full(cap - n, PAD_SENT))
        ra = np.concatenate(r_parts)
        da = np.concatenate(d_parts)
        # token s -> (partition s%128, column s//128)
        idx32 = ra.reshape(TOTC, 128).T.astype(np.int32)
        drel = da.reshape(TOTC, 128).T.astype(np.float32)
        per_core.append((np.ascontiguousarray(idx32),
                         np.ascontiguousarray(drel)))
    return per_core, [int(t) for t in tbs], perms


def _build_program(tbs):
    nc = bacc.Bacc(None, target_bir_lowering=False)
    dt = mybir.dt
    P = nc.declare_dram_parameter
    TOTC = sum(tbs)
    xT = P("xT", [IN_CH, NS], dt.float32, isOutput=False)
    w1p = P("w1p", [128, 128], dt.float32, isOutput=False)
    w2 = P("w2", [HID, HID], dt.float32, isOutput=False)
    wh1 = P("wh1", [HID, HID2], dt.float32, isOutput=False)
    wh2 = P("wh2", [HID2, OUT], dt.float32, isOutput=False)
    b1f = P("b1f", [128, HID], dt.float32, isOutput=False)
    b2f = P("b2f", [128, HID], dt.float32, isOutput=False)
    bh1 = P("bh1", [HID2, 1], dt.float32, isOutput=False)
    bh2 = P("bh2", [OUT, 1], dt.float32, isOutput=False)
    dinvP = P("dinvP", [128, NTILE], dt.float32, isOutput=False)
    idxP = P("idxP", [128, TOTC], dt.int32, isOutput=False)
    dstrelP = P("dstrelP", [128, TOTC], dt.float32, isOutput=False)
    iotaP = P("iotaP", [128, 1024], dt.float32, isOutput=False)
    outT = P("outT", [OUT, NS], dt.float32, isOutput=True)

    slice_d = [nc.dram_tensor(f"slice{l}", [NS, HID], dt.bfloat16) for l in (1, 2)]
    table_d = [nc.dram_tensor(f"table{l}", [8 * NS, HID], dt.bfloat16)
               for l in (1, 2)]

    iseq = mybir.AluOpType.is_equal
    relu = mybir.ActivationFunctionType.Relu
    copyf = mybir.ActivationFunctionType.Copy

    # block -> column range
    boff = np.cumsum([0] + list(tbs))

    with TileContext(nc) as tc:
        with tc.tile_pool(name="const", bufs=1) as cp, \
             tc.tile_pool(name="acc", bufs=1) as ap_, \
             tc.tile_pool(name="gath", bufs=16) as gp, \
             tc.tile_pool(name="work", bufs=3) as wp, \
             tc.tile_pool(name="m8", bufs=6) as mp, \
             tc.tile_pool(name="pst", bufs=2, space="PSUM") as ptp, \
             tc.tile_pool(name="psa", bufs=4, space="PSUM") as pap, \
             tc.tile_pool(name="psg", bufs=2, space="PSUM") as pgp:
            w1sb = cp.tile([128, 128], dt.float32)
            nc.sync.dma_start(out=w1sb[:], in_=w1p[:])
            w2sb = cp.tile([HID, HID], dt.float32)
            nc.sync.dma_start(out=w2sb[:], in_=w2[:])
            wh1sb = cp.tile([HID, HID2], dt.float32)
            nc.sync.dma_start(out=wh1sb[:], in_=wh1[:])
            wh2sb = cp.tile([HID2, OUT], dt.float32)
            nc.sync.dma_start(out=wh2sb[:], in_=wh2[:])
            b1sb = cp.tile([128, HID], dt.float32)
            nc.sync.dma_start(out=b1sb[:], in_=b1f[:])
            b2sb = cp.tile([128, HID], dt.float32)
            nc.sync.dma_start(out=b2sb[:], in_=b2f[:])
            bh1sb = cp.tile([HID2, 1], dt.float32)
            nc.sync.dma_start(out=bh1sb[:], in_=bh1[:])
            bh2sb = cp.tile([OUT, 1], dt.float32)
            nc.sync.dma_start(out=bh2sb[:], in_=bh2[:])
            dsb = cp.tile([128, NTILE], dt.float32)
            nc.sync.dma_start(out=dsb[:], in_=dinvP[:])
            ident = cp.tile([128, 128], dt.float32)
            make_identity(nc, ident[:])
            iotab = cp.tile([128, 1024], dt.bfloat16)
            iotaf = cp.tile([128, 1024], dt.float32)
            nc.sync.dma_start(out=iotaf[:], in_=iotaP[:])
            nc.vector.tensor_copy(iotab[:], iotaf[:])
            drb = cp.tile([128, TOTC], dt.bfloat16)
            drf = cp.tile([128, TOTC], dt.float32)
            nc.sync.dma_start(out=drf[:], in_=dstrelP[:])
            nc.vector.tensor_copy(drb[:], drf[:])
            idxsb = cp.tile([128, TOTC], dt.int32)
            nc.sync.dma_start(out=idxsb[:], in_=idxP[:])

            acc = ap_.tile([128, NBLK * HID], dt.float32)
            hwself = ap_.tile([128, NTILE * HID], dt.float32)

            def gemm1():
                for m in range(NTILE):
                    mc = slice(m * 128, (m + 1) * 128)
                    xa = wp.tile([128, 128], dt.float32, tag="xa")
                    nc.sync.dma_start(out=xa[:], in_=xT[0:128, mc])
                    xb = wp.tile([128, 128], dt.float32, tag="xb")
                    nc.sync.dma_start(out=xb[:], in_=xT[128:256, mc])
                    ps = pgp.tile([128, HID], dt.float32, tag="ps")
                    nc.tensor.matmul(ps[:], xa[:], w1sb[:, 0:HID],
                                     start=True, stop=False)
                    nc.tensor.matmul(ps[:], xb[:], w1sb[:, HID:128],
                                     start=False, stop=True)
                    hsl = hwself[:, m * HID:(m + 1) * HID]
                    nc.vector.tensor_scalar_mul(hsl, ps[:], dsb[:, m:m + 1])
                    sb16 = wp.tile([128, HID], dt.bfloat16, tag="sb16")
                    nc.scalar.activation(sb16[:], hsl, copyf)
                    nc.sync.dma_start(out=slice_d[0][mc, :], in_=sb16[:])

            def aggregate(layer):
                tbl = table_d[layer]
                # block schedule per column
                col_block = []
                for b in range(NBLK):
                    col_block += [b] * tbs[b]
                for c in range(TOTC):
                    g = gp.tile([128, HID], dt.bfloat16, tag="g")
                    nc.gpsimd.indirect_dma_start(
                        out=g[:],
                        out_offset=None,
                        in_=tbl[:],
                        in_offset=bass.IndirectOffsetOnAxis(
                            ap=idxsb[:, c:c + 1], axis=0),
                    )
                    if c % 8 == 0:
                        m8 = mp.tile([128, 1024], dt.bfloat16, tag="m8")
                        nw = min(8, TOTC - c)
                        d3b = drb[:, c:c + nw] \
                            .rearrange("p (k o) -> p k o", o=1) \
                            .broadcast_to((128, nw, 128))
                        nc.vector.tensor_tensor(
                            m8[:, 0:nw * 128].rearrange(
                                "p (k o) -> p k o", o=128),
                            iotab[:, 0:nw * 128].rearrange(
                                "p (k o) -> p k o", o=128),
                            d3b, iseq)
                    b = col_block[c]
                    start = c == boff[b]
                    stop = c == boff[b + 1] - 1
                    if start:
                        agg_t = pap.tile([128, HID], dt.float32, tag="agg")
                        cur = agg_t
                    nc.tensor.matmul(
                        cur[:],
                        m8[:, (c % 8) * 128:(c % 8 + 1) * 128],
                        g[:],
                        start=start, stop=stop)
                    if stop:
                        nc.vector.tensor_copy(
                            acc[:, b * HID:(b + 1) * HID], cur[:])

            def pointwise(layer):
                for m in range(NTILE):
                    mc = slice(m * 128, (m + 1) * 128)
                    hsl = hwself[:, m * HID:(m + 1) * HID]
                    s = wp.tile([128, HID], dt.float32, tag="s")
                    if m < NBLK:
                        nc.vector.tensor_add(
                            s[:], acc[:, m * HID:(m + 1) * HID], hsl)
                    else:
                        nc.vector.tensor_copy(s[:], hsl)
                    nc.vector.tensor_scalar_mul(s[:], s[:], dsb[:, m:m + 1])
                    nc.vector.tensor_add(s[:], s[:],
                                         b1sb[:] if layer == 0 else b2sb[:])
                    h = wp.tile([128, HID], dt.float32, tag="h")
                    nc.scalar.activation(h[:], s[:], relu)
                    pt = ptp.tile([128, 128], dt.float32, tag="tp")
                    nc.tensor.transpose(pt[0:HID, :], h[:], ident[:])
                    ht = wp.tile([HID, 128], dt.float32, tag="ht")
                    nc.scalar.activation(ht[:], pt[0:HID, :], copyf)
                    if layer == 0:
                        ps2 = pgp.tile([128, HID], dt.float32, tag="ps")
                        nc.tensor.matmul(ps2[:], ht[:], w2sb[:],
                                         start=True, stop=True)
                        nc.vector.tensor_scalar_mul(hsl, ps2[:],
                                                    dsb[:, m:m + 1])
                        sb16 = wp.tile([128, HID], dt.bfloat16, tag="sb16b")
                        nc.scalar.activation(sb16[:], hsl, copyf)
                        nc.sync.dma_start(out=slice_d[1][mc, :], in_=sb16[:])
                    else:
                        pz = ptp.tile([128, 128], dt.float32, tag="tp")
                        nc.tensor.matmul(pz[0:HID2, :], wh1sb[:], ht[:],
                                         start=True, stop=True)
                        zb = wp.tile([HID2, 128], dt.float32, tag="zb")
                        nc.scalar.activation(zb[:], pz[0:HID2, :], relu,
                                             bias=bh1sb[:])
                        po = ptp.tile([128, 128], dt.float32, tag="tp")
                        nc.tensor.matmul(po[0:OUT, :], wh2sb[:], zb[:],
                                         start=True, stop=True)
                        ob = wp.tile([OUT, 128], dt.float32, tag="ob")
                        nc.vector.tensor_scalar_add(ob[:], po[0:OUT, :],
                                                    bh2sb[:])
                        nc.sync.dma_start(out=outT[:, mc], in_=ob[:])

            gemm1()
            nc.gpsimd.collective_compute(
                "AllGather", mybir.AluOpType.bypass,
                replica_groups=[list(range(8))],
                ins=[slice_d[0][:]], outs=[table_d[0][:]])
            aggregate(0)
            pointwise(0)
            nc.gpsimd.collective_compute(
                "AllGather", mybir.AluOpType.bypass,
                replica_groups=[list(range(8))],
                ins=[slice_d[1][:]], outs=[table_d[1][:]])
            aggregate(1)
            pointwise(1)

    nc.finalize()
    return nc


def kernel(x, edge_index, W1, b1, W2, b2, Wh1, bh1, Wh2, bh2, _trace=False):
    x = np.asarray(x, np.float32)
    src = np.asarray(edge_index[0], np.int64)
    dst = np.asarray(edge_index[1], np.int64)

    per_core, tbs, perms = _build_schedule(src, dst)
    sig = tuple(tbs)
    if sig not in _compiled:
        _compiled[sig] = _build_program(tbs)
    nc = _compiled[sig]

    deg = np.bincount(dst, minlength=N).astype(np.float64) + 1.0
    dinv = (1.0 / np.sqrt(deg)).astype(np.float32)

    W1 = np.asarray(W1, np.float32)
    w1p = np.concatenate([W1[:128], W1[128:]], axis=1)
    b1f = np.tile(np.asarray(b1, np.float32)[None, :], (128, 1))
    b2f = np.tile(np.asarray(b2, np.float32)[None, :], (128, 1))
    bh1c = np.asarray(bh1, np.float32)[:, None]
    bh2c = np.asarray(bh2, np.float32)[:, None]
    iota = np.tile(np.arange(128, dtype=np.float32)[None, :], (128, 8))

    in_maps = []
    for c in range(8):
        idx32, dstrel = per_core[c]
        xs = np.zeros((NS, IN_CH), np.float32)
        xs[perms[c]] = x[c * NS_RAW:(c + 1) * NS_RAW]
        dv = np.ones(NS, np.float32)
        dv[perms[c]] = dinv[c * NS_RAW:(c + 1) * NS_RAW]
        in_maps.append({
            "xT": np.ascontiguousarray(xs.T),
            "w1p": np.ascontiguousarray(w1p),
            "w2": np.asarray(W2, np.float32),
            "wh1": np.asarray(Wh1, np.float32),
            "wh2": np.asarray(Wh2, np.float32),
            "b1f": b1f, "b2f": b2f, "bh1": bh1c, "bh2": bh2c,
            "dinvP": np.ascontiguousarray(dv.reshape(NTILE, 128).T),
            "idxP": idx32,
            "dstrelP": dstrel,
            "iotaP": iota,
        })

    res = run_bass_kernel_spmd(nc, in_maps, list(range(8)), trace=_trace)
    out = np.empty((N, OUT), np.float32)
    for c in range(8):
        out[c * NS_RAW:(c + 1) * NS_RAW] = res.results[c]["outT"].T[perms[c]]
    if _trace:
        kernel.last_results = res
    return out



# revision 6
# speedup vs baseline: 1.2871x; 1.2871x over previous
"""GCN (2-layer + MLP head) on 8 NeuronCores — v3: indirect-DMA gather +
matmul aggregation.

Per core (nodes dst-sharded, 12500 real / 12800 padded):
  GEMM: hw = (x @ W1) * dinv  (node-major, fp32, SBUF-resident self table)
  slice [12800, 64] bf16 -> AllGather -> table [102400, 64] bf16 in DRAM.
  Edge tokens dst-sorted, grouped by dst-block (128 dsts), each block's
  token count padded to a cross-core-common multiple of 128.
  gpsimd.indirect_dma_start gathers 8192 tokens/call from the full table
  (HW DGE, int32 row indices, token-major out [128, 64, 64] bf16).
  Aggregation: per token-tile, M[t,d] = (dstrel[t]==d) built on DVE
  (is_equal vs iota, bf16), PE matmul M^T @ g accumulates a dst-block's
  tiles in PSUM; one fold per block into SBUF acc (fp32).
  Pointwise: h = relu((acc + hw_self)*dinv + b); layer-2 GEMM via PE
  transpose of h; head MLP per tile; out [2, 12800] per core.
"""
import numpy as np

import concourse.bacc as bacc
import concourse.mybir as mybir
from concourse import bass
from concourse.tile import TileContext
from concourse.bass_utils import run_bass_kernel_spmd
from concourse.masks import make_identity

N = 100000
NS_RAW = 12500
NS = 12800
NTILE = NS // 128          # 100
NBLK = 98                  # blocks containing real dsts
IN_CH, HID, HID2, OUT = 256, 64, 32, 2
NT_CALL = 64               # gather-call size in token-tiles (8192 tokens)
PAD_SENT = -1000.0

_compiled = {}


def _build_schedule(src, dst):
    """Token schedule, shape-equalized across cores.

    Returns (per_core, tbs):
      per_core[c] = (idx32 [128, TOTC] int32 table-row indices,
                     dstrel [128, TOTC] f32 dst-local-in-block / PAD_SENT)
      tbs = [tiles per block] (common across cores), sum = TOTC
    """
    core = dst // NS_RAW
    dstl = (dst % NS_RAW).astype(np.int64)
    src_core = src // NS_RAW
    src_local = (src % NS_RAW).astype(np.int64)

    # Per-core node permutation: sort own dsts by in-degree desc, deal
    # round-robin into the 98 blocks so per-block token counts are nearly
    # equal (minimizes cross-core common padding). perm[old_local] = new.
    perms = []
    for c in range(8):
        degc = np.bincount(dstl[core == c], minlength=NS_RAW)
        order = np.argsort(-degc, kind="stable")
        ranks = np.arange(NS_RAW)
        newpos = (ranks % NBLK) * 128 + ranks // NBLK
        perm = np.empty(NS_RAW, np.int64)
        perm[order] = newpos
        perms.append(perm)
    permsA = np.stack(perms)                       # [8, NS_RAW]

    dstp = permsA[core, dstl]                      # permuted dst-local
    rows = src_core * NS + permsA[src_core, src_local]  # permuted table row
    blk = dstp // 128

    cells = {}
    ntile = np.zeros((8, NBLK), np.int64)
    for c in range(8):
        m = core == c
        o = np.argsort(dstp[m], kind="stable")
        rr, dd, bb = rows[m][o], dstp[m][o], blk[m][o]
        bounds = np.searchsorted(bb, np.arange(NBLK + 1))
        cells[c] = (bounds, rr, dd)
        ntile[c] = np.ceil((bounds[1:] - bounds[:-1]) / 128).astype(np.int64)

    tbs = ntile.max(axis=0)            # tiles per block, common
    assert tbs.min() >= 1
    TOTC = int(tbs.sum())

    per_core = []
    for c in range(8):
        bounds, rr, dd = cells[c]
        r_parts, d_parts = [], []
        for b in range(NBLK):
            lo, hi = bounds[b], bounds[b + 1]
            n = hi - lo
            cap = int(tbs[b]) * 128
            r_parts.append(rr[lo:hi])
            d_parts.append(dd[lo:hi] - 128 * b)
            if cap > n:
                r_parts.append(np.zeros(cap - n, np.int64))
                d_parts.append(np.full(cap - n, PAD_SENT))
        ra = np.concatenate(r_parts)
        da = np.concatenate(d_parts)
        # token s -> (partition s%128, column s//128)
        idx32 = ra.reshape(TOTC, 128).T.astype(np.int32)
        drel = da.reshape(TOTC, 128).T.astype(np.float32)
        per_core.append((np.ascontiguousarray(idx32),
                         np.ascontiguousarray(drel)))
    return per_core, [int(t) for t in tbs], perms


def _build_program(tbs):
    nc = bacc.Bacc(None, target_bir_lowering=False)
    dt = mybir.dt
    P = nc.declare_dram_parameter
    TOTC = sum(tbs)
    xT = P("xT", [IN_CH, NS], dt.float32, isOutput=False)
    w1p = P("w1p", [128, 128], dt.float32, isOutput=False)
    w2 = P("w2", [HID, HID], dt.float32, isOutput=False)
    wh1 = P("wh1", [HID, HID2], dt.float32, isOutput=False)
    wh2 = P("wh2", [HID2, OUT], dt.float32, isOutput=False)
    b1f = P("b1f", [128, HID], dt.float32, isOutput=False)
    b2f = P("b2f", [128, HID], dt.float32, isOutput=False)
    bh1 = P("bh1", [HID2, 1], dt.float32, isOutput=False)
    bh2 = P("bh2", [OUT, 1], dt.float32, isOutput=False)
    dinvP = P("dinvP", [128, NTILE], dt.float32, isOutput=False)
    idxP = P("idxP", [128, TOTC], dt.int32, isOutput=False)
    dstrelP = P("dstrelP", [128, TOTC], dt.float32, isOutput=False)
    iotaP = P("iotaP", [128, 1024], dt.float32, isOutput=False)
    outT = P("outT", [OUT, NS], dt.float32, isOutput=True)

    slice_d = [nc.dram_tensor(f"slice{l}", [NS, HID], dt.bfloat16) for l in (1, 2)]
    table_d = [nc.dram_tensor(f"table{l}", [8 * NS, HID], dt.bfloat16)
               for l in (1, 2)]

    iseq = mybir.AluOpType.is_equal
    relu = mybir.ActivationFunctionType.Relu
    copyf = mybir.ActivationFunctionType.Copy

    # block -> column range
    boff = np.cumsum([0] + list(tbs))

    with TileContext(nc) as tc:
        with tc.tile_pool(name="const", bufs=1) as cp, \
             tc.tile_pool(name="acc", bufs=1) as ap_, \
             tc.tile_pool(name="gath", bufs=16) as gp, \
             tc.tile_pool(name="work", bufs=3) as wp, \
             tc.tile_pool(name="m8", bufs=6) as mp, \
             tc.tile_pool(name="pst", bufs=2, space="PSUM") as ptp, \
             tc.tile_pool(name="psa", bufs=4, space="PSUM") as pap, \
             tc.tile_pool(name="psg", bufs=2, space="PSUM") as pgp:
            w1sb = cp.tile([128, 128], dt.float32)
            nc.sync.dma_start(out=w1sb[:], in_=w1p[:])
            w2sb = cp.tile([HID, HID], dt.float32)
            nc.sync.dma_start(out=w2sb[:], in_=w2[:])
            wh1sb = cp.tile([HID, HID2], dt.float32)
            nc.sync.dma_start(out=wh1sb[:], in_=wh1[:])
            wh2sb = cp.tile([HID2, OUT], dt.float32)
            nc.sync.dma_start(out=wh2sb[:], in_=wh2[:])
            b1sb = cp.tile([128, HID], dt.float32)
            nc.sync.dma_start(out=b1sb[:], in_=b1f[:])
            b2sb = cp.tile([128, HID], dt.float32)
            nc.sync.dma_start(out=b2sb[:], in_=b2f[:])
            bh1sb = cp.tile([HID2, 1], dt.float32)
            nc.sync.dma_start(out=bh1sb[:], in_=bh1[:])
            bh2sb = cp.tile([OUT, 1], dt.float32)
            nc.sync.dma_start(out=bh2sb[:], in_=bh2[:])
            dsb = cp.tile([128, NTILE], dt.float32)
            nc.sync.dma_start(out=dsb[:], in_=dinvP[:])
            ident = cp.tile([128, 128], dt.float32)
            make_identity(nc, ident[:])
            iotab = cp.tile([128, 1024], dt.bfloat16)
            iotaf = cp.tile([128, 1024], dt.float32)
            nc.sync.dma_start(out=iotaf[:], in_=iotaP[:])
            nc.vector.tensor_copy(iotab[:], iotaf[:])
            drb = cp.tile([128, TOTC], dt.bfloat16)
            drf = cp.tile([128, TOTC], dt.float32)
            nc.sync.dma_start(out=drf[:], in_=dstrelP[:])
            nc.vector.tensor_copy(drb[:], drf[:])
            idxsb = cp.tile([128, TOTC], dt.int32)
            nc.sync.dma_start(out=idxsb[:], in_=idxP[:])

            acc = ap_.tile([128, NBLK * HID], dt.float32)
            hwself = ap_.tile([128, NTILE * HID], dt.float32)

            def gemm1():
                for m in range(NTILE):
                    mc = slice(m * 128, (m + 1) * 128)
                    xa = wp.tile([128, 128], dt.float32, tag="xa")
                    nc.sync.dma_start(out=xa[:], in_=xT[0:128, mc])
                    xb = wp.tile([128, 128], dt.float32, tag="xb")
                    nc.sync.dma_start(out=xb[:], in_=xT[128:256, mc])
                    ps = pgp.tile([128, HID], dt.float32, tag="ps")
                    nc.tensor.matmul(ps[:], xa[:], w1sb[:, 0:HID],
                                     start=True, stop=False)
                    nc.tensor.matmul(ps[:], xb[:], w1sb[:, HID:128],
                                     start=False, stop=True)
                    hsl = hwself[:, m * HID:(m + 1) * HID]
                    nc.vector.tensor_scalar_mul(hsl, ps[:], dsb[:, m:m + 1])
                    sb16 = wp.tile([128, HID], dt.bfloat16, tag="sb16")
                    nc.scalar.activation(sb16[:], hsl, copyf)
                    nc.sync.dma_start(out=slice_d[0][mc, :], in_=sb16[:])

            def aggregate(layer):
                tbl = table_d[layer]
                # block schedule per column
                col_block = []
                for b in range(NBLK):
                    col_block += [b] * tbs[b]
                for c in range(TOTC):
                    g = gp.tile([128, HID], dt.bfloat16, tag="g")
                    nc.gpsimd.indirect_dma_start(
                        out=g[:],
                        out_offset=None,
                        in_=tbl[:],
                        in_offset=bass.IndirectOffsetOnAxis(
                            ap=idxsb[:, c:c + 1], axis=0),
                    )
                    if c % 8 == 0:
                        m8 = mp.tile([128, 1024], dt.bfloat16, tag="m8")
                        nw = min(8, TOTC - c)
                        d3b = drb[:, c:c + nw] \
                            .rearrange("p (k o) -> p k o", o=1) \
                            .broadcast_to((128, nw, 128))
                        nc.vector.tensor_tensor(
                            m8[:, 0:nw * 128].rearrange(
                                "p (k o) -> p k o", o=128),
                            iotab[:, 0:nw * 128].rearrange(
                                "p (k o) -> p k o", o=128),
                            d3b, iseq)
                    b = col_block[c]
                    start = c == boff[b]
                    stop = c == boff[b + 1] - 1
                    if start:
                        agg_t = pap.tile([128, HID], dt.float32, tag="agg")
                        cur = agg_t
                    nc.tensor.matmul(
                        cur[:],
                        m8[:, (c % 8) * 128:(c % 8 + 1) * 128],
                        g[:],
                        start=start, stop=stop)
                    if stop:
                        nc.vector.tensor_copy(
                            acc[:, b * HID:(b + 1) * HID], cur[:])

            def pointwise(layer):
                for m in range(NTILE):
                    mc = slice(m * 128, (m + 1) * 128)
                    hsl = hwself[:, m * HID:(m + 1) * HID]
                    s = wp.tile([128, HID], dt.float32, tag="s")
                    if m < NBLK:
                        nc.vector.tensor_add(
                            s[:], acc[:, m * HID:(m + 1) * HID], hsl)
                    else:
                        nc.vector.tensor_copy(s[:], hsl)
                    nc.vector.tensor_scalar_mul(s[:], s[:], dsb[:, m:m + 1])
                    nc.vector.tensor_add(s[:], s[:],
                                         b1sb[:] if layer == 0 else b2sb[:])
                    h = wp.tile([128, HID], dt.float32, tag="h")
                    nc.scalar.activation(h[:], s[:], relu)
                    pt = ptp.tile([128, 128], dt.float32, tag="tp")
                    nc.tensor.transpose(pt[0:HID, :], h[:], ident[:])
                    ht = wp.tile([HID, 128], dt.float32, tag="ht")
                    nc.scalar.activation(ht[:], pt[0:HID, :], copyf)
                    if layer == 0:
                        ps2 = pgp.tile([128, HID], dt.float32, tag="ps")
                        nc.tensor.matmul(ps2[:], ht[:], w2sb[:],
                                         start=True, stop=True)
                        nc.vector.tensor_scalar_mul(hsl, ps2[:],
                                                    dsb[:, m:m + 1])
                        sb16 = wp.tile([128, HID], dt.bfloat16, tag="sb16b")
                        nc.scalar.activation(sb16[:], hsl, copyf)
                        nc.sync.dma_start(out=slice_d[1][mc, :], in_=sb16[:])
                    else:
                        pz = ptp.tile([128, 128], dt.float32, tag="tp")
                        nc.tensor.matmul(pz[0:HID2, :], wh1sb[:], ht[:],
                                         start=True, stop=True)
                        zb = wp.tile([HID2, 128], dt.float32, tag="zb")
                        nc.scalar.activation(zb[:], pz[0:HID2, :], relu,
                                             bias=bh1sb[:])
                        po = ptp.tile([128, 128], dt.float32, tag="tp")
                        nc.tensor.matmul(po[0:OUT, :], wh2sb[:], zb[:],
                                         start=True, stop=True)
                        ob = wp.tile([OUT, 128], dt.float32, tag="ob")
                        nc.vector.tensor_scalar_add(ob[:], po[0:OUT, :],
                                                    bh2sb[:])
                        nc.sync.dma_start(out=outT[:, mc], in_=ob[:])

            gemm1()
            nc.gpsimd.collective_compute(
                "AllGather", mybir.AluOpType.bypass,
                replica_groups=[list(range(8))],
                ins=[slice_d[0][:]], outs=[table_d[0][:]])
            aggregate(0)
            pointwise(0)
            nc.gpsimd.collective_compute(
                "AllGather", mybir.AluOpType.bypass,
                replica_groups=[list(range(8))],
                ins=[slice_d[1][:]], outs=[table_d[1][:]])
            aggregate(1)
            pointwise(1)

    nc.finalize()
    return nc


def kernel(x, edge_index, W1, b1, W2, b2, Wh1, bh1, Wh2, bh2, _trace=False):
    x = np.asarray(x, np.float32)
    src = np.asarray(edge_index[0], np.int64)
    dst = np.asarray(edge_index[1], np.int64)

    per_core, tbs, perms = _build_schedule(src, dst)
    sig = tuple(tbs)
    if sig not in _compiled:
        _compiled[sig] = _build_program(tbs)
    nc = _compiled[sig]

    deg = np.bincount(dst, minlength=N).astype(np.float64) + 1.0
    dinv = (1.0 / np.sqrt(deg)).astype(np.float32)

    W1 = np.asarray(W1, np.float32)
    w1p = np.concatenate([W1[:128], W1[128:]], axis=1)
    b1f = np.tile(np.asarray(b1, np.float32)[None, :], (128, 1))
    b2f = np.tile(np.asarray(b2, np.float32)[None, :], (128, 1))
    bh1c = np.asarray(bh1, np.float32)[:, None]
    bh2c = np.asarray(bh2, np.float32)[:, None]
    iota = np.tile(np.arange(128, dtype=np.float32)[None, :], (128, 8))

    in_maps = []
    for c in range(8):
        idx32, dstrel = per_core[c]
        xs = np.zeros((NS, IN_CH), np.float32)
        xs[perms[c]] = x[c * NS_RAW:(c + 1) * NS_RAW]
        dv = np.ones(NS, np.float32)
        dv[perms[c]] = dinv[c * NS_RAW:(c + 1) * NS_RAW]
        in_maps.append({
            "xT": np.ascontiguousarray(xs.T),
            "w1p": np.ascontiguousarray(w1p),
            "w2": np.asarray(W2, np.float32),
            "wh1": np.asarray(Wh1, np.float32),
            "wh2": np.asarray(Wh2, np.float32),
            "b1f": b1f, "b2f": b2f, "bh1": bh1c, "bh2": bh2c,
            "dinvP": np.ascontiguousarray(dv.reshape(NTILE, 128).T),
            "idxP": idx32,
            "dstrelP": dstrel,
            "iotaP": iota,
        })

    res = run_bass_kernel_spmd(nc, in_maps, list(range(8)), trace=_trace)
    out = np.empty((N, OUT), np.float32)
    for c in range(8):
        out[c * NS_RAW:(c + 1) * NS_RAW] = res.results[c]["outT"].T[perms[c]]
    if _trace:
        kernel.last_results = res
    return out

